# revision 14
# baseline (speedup 1.0000x reference)
"""Bahdanau temporal attention on 8 Trainium2 NeuronCores.

Full-input contract: kernel(**inputs) takes the unsharded numpy arrays
(query (32,1024), keys (32,4096,1024), Wq (1024,512), Wk (1024,512),
v (512,)) and returns the full output (32,1,1024) float32.

Sharding: data-parallel over batch. Each of the 8 cores processes 4
batches; Wq/Wk/v are replicated. No collectives.

Per-core algorithm (B_loc=4, S=4096, H=1024, A=512):
  q_t    = query @ Wq                 (B_loc, A)
  k_t    = keys @ Wk                  (B_loc, S, A)
  energy = v . tanh(q_t + k_t)        (B_loc, S)
  w      = exp(energy)   (unnormalized; |energy| <= |v|_1 so exp cannot
                          overflow fp32 and no max-subtraction is needed)
  ctx    = (w @ keys) / sum(w)        (B_loc, H)

Pipeline per 512-row S-tile (s = r*128 + p), 3 software stages:
  stage A (front): SWDGE keys DMA casting f32->bf16 in flight, then
    xbar DMA-transpose -> keysT [h' (part), r, hc, s'],
  stage B (proj):
    - PE: k_t^T = Wk^T @ keys^T (bf16, f32 PSUM, N=512 streams),
    - ACT: T = tanh(k_t^T + q_t^T), q_t as per-partition bias,
    - PE: energy16 = v_rep16^T @ T -> psum [16, 512] (v_rep16 has 16
      identical columns of v, so all 16 psum rows hold the energy row),
    - ACT: w16 = exp(energy16) with accum_out -> per-tile softmax
      normalizer Z (no matmuls or max-subtraction needed),
    - xbar w16 [16, 512] -> wT4 [s' (part), r, 16] (same proven form as
      the q/v transposes),
  stage C (ctx, one stage later so the w xbar hides under stage B of
  the next tile):
    - PE: ctx [1, H] += wT4[:, r, 0].T @ keys_nat[:, r, :] -- full
      K=128 contraction, batched over r and the two H halves, psum
      accumulated across all 8 S-tiles of a batch.
Finalize per batch: Z = sum of per-tile normalizers (DVE adds),
reciprocal, scale the two psum halves, DMA out.
"""

import os
import sys

if "/opt/trn_rl_repo" not in sys.path:
    sys.path.insert(0, "/opt/trn_rl_repo")

import numpy as np

import concourse.bass as bass
import concourse.tile as tile
from concourse import bacc
from concourse import mybir
from concourse.bass_utils import run_bass_kernel_spmd

F32 = mybir.dt.float32
BF16 = mybir.dt.bfloat16

N_CORES = 8
B, S, H, A = 32, 4096, 1024, 512
B_LOC = B // N_CORES          # 4 batches per core
ST = 512                      # S-tile rows
N_ST = S // ST                # 8 S-tiles per batch
P = 128                       # partitions
HC = H // P                   # 8 contraction chunks
AC = A // P                   # 4 a-chunks
R = ST // P                   # 4 s-rows per partition (s = r*128 + p)

XBAR_SPLIT = int(os.environ.get("K_XBAR_SPLIT", "2"))  # xbar calls per keys tile
SWDGE_CAST = os.environ.get("K_SWDGE_CAST", "1") == "1"  # cast f32->bf16 in DMA

Tanh = mybir.ActivationFunctionType.Tanh
Exp = mybir.ActivationFunctionType.Exp


def build_bass():
    nc = bacc.Bacc()

    d_query = nc.declare_dram_parameter("query", [B_LOC, H], F32, isOutput=False)
    d_keys = nc.declare_dram_parameter("keys", [B_LOC, S, H], F32, isOutput=False)
    d_wq = nc.declare_dram_parameter("Wq", [H, A], F32, isOutput=False)
    d_wk = nc.declare_dram_parameter("Wk", [H, A], F32, isOutput=False)
    d_v = nc.declare_dram_parameter("v", [A], F32, isOutput=False)
    d_out = nc.declare_dram_parameter("out", [B_LOC, H], F32, isOutput=True)

    from contextlib import ExitStack

    with tile.TileContext(nc) as tc, ExitStack() as ctx:
        build_kernel_body(tc, d_query, d_keys, d_wq, d_wk, d_v, d_out, ctx)
    nc.compile()
    return nc


def build_kernel_body(tc, d_query, d_keys, d_wq, d_wk, d_v, d_out, ctx):
    nc = tc.nc

    consts = ctx.enter_context(tc.tile_pool(name="consts", bufs=1))
    keyp = ctx.enter_context(tc.tile_pool(name="keyp", bufs=9))
    keytp = ctx.enter_context(tc.tile_pool(name="keytp", bufs=5))
    tp = ctx.enter_context(tc.tile_pool(name="tp", bufs=3))
    wp = ctx.enter_context(tc.tile_pool(name="wp", bufs=5))
    wtp = ctx.enter_context(tc.tile_pool(name="wtp", bufs=5))
    bp = ctx.enter_context(tc.tile_pool(name="bp", bufs=2))
    smalls = ctx.enter_context(tc.tile_pool(name="smalls", bufs=2))
    pp_kt = ctx.enter_context(tc.tile_pool(name="pp_kt", bufs=3, space="PSUM"))
    pp_e = ctx.enter_context(tc.tile_pool(name="pp_e", bufs=1, space="PSUM"))
    pp_ctx = ctx.enter_context(tc.tile_pool(name="pp_ctx", bufs=4, space="PSUM"))

    # ---- constants ----
    # Wk in bf16, laid out [h' (part), hc, a]; SWDGE casts f32 -> bf16
    wk_bf = consts.tile([P, HC, A], BF16)
    nc.gpsimd.dma_start(out=wk_bf, in_=d_w_rearr(d_wk))
    wq_sb = consts.tile([P, HC, A], BF16)
    nc.gpsimd.dma_start(out=wq_sb, in_=d_w_rearr(d_wq))

    # v: load f32, DVE-cast into row 0 of a 16-row tile (single-producer
    # funnel so the xbar transpose carries only one wait), then xbar.
    v_f32 = consts.tile([1, A], F32)
    nc.gpsimd.dma_start(out=v_f32, in_=d_v[None, :])
    v16 = consts.tile([16, A], BF16)
    nc.vector.memset(v16, 0.0)
    nc.vector.tensor_copy(v16[0:1, :], v_f32)
    vT16 = consts.tile([P, AC, 16], BF16)
    nc.sync.dma_start(out=vT16, in_=v16, transpose=True)

    # query: same funnel pattern
    q_f32 = consts.tile([B_LOC, H], F32)
    nc.gpsimd.dma_start(out=q_f32, in_=d_query[:, :])
    q16 = consts.tile([16, H], BF16)
    nc.vector.memset(q16, 0.0)
    nc.vector.tensor_copy(q16[0:B_LOC, :], q_f32)
    qT16 = consts.tile([P, HC, 16], BF16)
    nc.sync.dma_start(out=qT16, in_=q16, transpose=True)

    # q_t = query @ Wq : psum (16, A), accumulate over hc
    ps_qt = pp_e.tile([16, A], F32, tag="e")
    for hc in range(HC):
        nc.tensor.matmul(
            ps_qt,
            lhsT=qT16[:, hc, :],
            rhs=wq_sb[:, hc, :],
            start=(hc == 0),
            stop=(hc == HC - 1),
        )
    qt16 = consts.tile([16, A], BF16)
    nc.vector.memset(qt16, 0.0)
    nc.vector.tensor_copy(qt16[0:B_LOC, :], ps_qt[0:B_LOC, :])
    # xbar -> qtT16 (128, AC, 16); tanh bias per (ac, b) = qtT16[:, ac, b]
    qtT16 = consts.tile([P, AC, 16], BF16)
    nc.sync.dma_start(out=qtT16, in_=qt16, transpose=True)

    # v_rep16[p, ac, m] = v[ac*128 + p] for m in 0..16: stationary operand
    # whose 16 identical columns replicate the energy row over 16 psum rows
    ones16 = consts.tile([P, 16], BF16)
    nc.vector.memset(ones16, 1.0)
    vTf = consts.tile([P, AC], F32)
    nc.vector.tensor_copy(vTf, vT16[:, :, 0])
    v_rep16 = consts.tile([P, AC, 16], BF16)
    for ac in range(AC):
        nc.vector.tensor_scalar_mul(v_rep16[:, ac, :], ones16, vTf[:, ac : ac + 1])

    # ---- main loop: 3-stage pipelined emission ----
    iters = [(b, st) for b in range(B_LOC) for st in range(N_ST)]
    loads = {}
    front = {}
    w16s = {}
    wts = {}
    ctx_psums = {}
    zparts = {}

    def stage_load(b, st):
        # keys tile [s' (part), r, h] with s = r*128 + p
        src = d_keys[b, st * ST : (st + 1) * ST, :].rearrange("(r p) h -> p r h", p=P)
        if SWDGE_CAST:
            keys_bf = keyp.tile([P, R, H], BF16, tag="keys")
            nc.gpsimd.dma_start(out=keys_bf, in_=src)
        else:
            keys_f32 = keyp.tile([P, R, H], F32, tag="keysf")
            nc.scalar.dma_start(out=keys_f32, in_=src)
            keys_bf = keyp.tile([P, R, H], BF16, tag="keys")
            nc.vector.tensor_copy(keys_bf, keys_f32)
        return keys_bf

    def stage_xpose(b, st):
        # xbar: keysT[h', r, hc, s'] = keys_bf[s', r, hc*128 + h']
        # NOTE: all DMA-transposes must stay on ONE queue -- concurrent
        # xbar transposes from different HWDGE queues corrupt each other
        # (shared xbar HW; the framework only serializes per-queue).
        keys_bf = loads[(b, st)]
        keysT = keytp.tile([P, R, HC, P], BF16, tag="kT")
        if XBAR_SPLIT == 1:
            nc.sync.dma_start(out=keysT, in_=keys_bf, transpose=True)
        else:
            rr = R // XBAR_SPLIT
            for j in range(XBAR_SPLIT):
                nc.sync.dma_start(
                    out=keysT[:, j * rr : (j + 1) * rr, :, :],
                    in_=keys_bf[:, j * rr : (j + 1) * rr, :],
                    transpose=True,
                )
        return keys_bf, keysT

    def stage_proj(b, st):
        keys_bf, keysT = front[(b, st)]
        if st == 0:
            z_parts = bp.tile([16, N_ST], F32, tag="z")
            zparts[b] = z_parts
        z_parts = zparts[b]

        # projection + tanh: T[a' (part), ac, s]
        T_sb = tp.tile([P, AC, ST], BF16, tag="T")
        for ac in range(AC):
            ps_kt = pp_kt.tile([P, ST], F32, tag="kt")
            for hc in range(HC):
                nc.tensor.matmul(
                    ps_kt,
                    lhsT=wk_bf[:, hc, ac * P : (ac + 1) * P],
                    rhs=keysT[:, :, hc, :],
                    start=(hc == 0),
                    stop=(hc == HC - 1),
                )
            nc.scalar.activation(
                T_sb[:, ac, :],
                ps_kt,
                Tanh,
                bias=qtT16[:, ac, b : b + 1],
            )

        # energy row, replicated over 16 psum rows
        ps_e = pp_e.tile([16, ST], F32, tag="e")
        for ac in range(AC):
            nc.tensor.matmul(
                ps_e,
                lhsT=v_rep16[:, ac, :],
                rhs=T_sb[:, ac, :],
                start=(ac == 0),
                stop=(ac == AC - 1),
            )

        # w16 = exp(energy); accum_out = per-tile softmax normalizer
        w16 = wp.tile([16, ST], BF16, tag="w")
        nc.scalar.activation(w16, ps_e, Exp, accum_out=z_parts[:, st : st + 1])
        w16s[(b, st)] = w16

    def stage_wx(b, st):
        # w transpose: wT4[p, r, i] = w[r*128 + p] for any i. Emitted two
        # steps after its exp so the in-order transpose queue never
        # head-of-line blocks on it.
        w16 = w16s.pop((b, st))
        wT4 = wtp.tile([P, R, 16], BF16, tag="wT")
        nc.sync.dma_start(out=wT4, in_=w16, transpose=True)
        wts[(b, st)] = wT4

    def stage_ctx(b, st):
        keys_bf, _ = front.pop((b, st))
        wT4 = wts.pop((b, st))
        first = st == 0
        last = st == N_ST - 1
        if first:
            ps_c0 = pp_ctx.tile([1, 512], F32, tag="ctx")
            ps_c1 = pp_ctx.tile([1, 512], F32, tag="ctx")
            ctx_psums[b] = (ps_c0, ps_c1)
        ps_c0, ps_c1 = ctx_psums[b]

        for r in range(R):
            nc.tensor.matmul(
                ps_c0,
                lhsT=wT4[:, r, 0:1],
                rhs=keys_bf[:, r, 0:512],
                start=(first and r == 0),
                stop=(last and r == R - 1),
            )
            nc.tensor.matmul(
                ps_c1,
                lhsT=wT4[:, r, 0:1],
                rhs=keys_bf[:, r, 512:1024],
                start=(first and r == 0),
                stop=(last and r == R - 1),
            )
        if last:
            finalize_batch(b, ps_c0, ps_c1)

    def finalize_batch(b, ps_c0, ps_c1):
        z_parts = zparts.pop(b)
        ctx_psums.pop(b)
        # Z = sum of the 8 per-tile normalizers (row 0 carries the value)
        za = smalls.tile([16, 1], F32, tag="za")
        zb = smalls.tile([16, 1], F32, tag="zb")
        nc.vector.tensor_scalar_add(za, z_parts[:, 0:1], z_parts[:, 1:2])
        nc.vector.tensor_scalar_add(zb, z_parts[:, 2:3], z_parts[:, 3:4])
        nc.vector.tensor_scalar_add(za, za, z_parts[:, 4:5])
        nc.vector.tensor_scalar_add(zb, zb, z_parts[:, 5:6])
        nc.vector.tensor_scalar_add(za, za, z_parts[:, 6:7])
        nc.vector.tensor_scalar_add(zb, zb, z_parts[:, 7:8])
        nc.vector.tensor_scalar_add(za, za, zb[0:16, 0:1])
        rz = smalls.tile([1, 1], F32, tag="rz")
        nc.vector.reciprocal(rz, za[0:1, 0:1])
        out_sb = smalls.tile([1, H], F32, tag="out")
        nc.vector.tensor_scalar_mul(out_sb[0:1, 0:512], ps_c0, rz)
        nc.vector.tensor_scalar_mul(out_sb[0:1, 512:1024], ps_c1, rz)
        nc.gpsimd.dma_start(out=d_out[b : b + 1, :], in_=out_sb)

    n = len(iters)
    for i in range(n + 4):
        if i < n:
            loads[iters[i]] = stage_load(*iters[i])
        if 1 <= i <= n:
            front[iters[i - 1]] = stage_xpose(*iters[i - 1])
            loads.pop(iters[i - 1])
        if 2 <= i <= n + 1:
            stage_proj(*iters[i - 2])
        if 3 <= i <= n + 2:
            stage_wx(*iters[i - 3])
        if 4 <= i <= n + 3:
            stage_ctx(*iters[i - 4])


def d_w_rearr(d_w):
    # (H, A) dram -> [h' (part), hc, a] view
    return d_w.rearrange("(hc p) a -> p hc a", p=P)


_CACHED_NC = None


def _get_nc():
    global _CACHED_NC
    if _CACHED_NC is None:
        _CACHED_NC = build_bass()
    return _CACHED_NC


def kernel(query, keys, Wq, Wk, v):
    query = np.ascontiguousarray(np.asarray(query, dtype=np.float32))
    keys = np.ascontiguousarray(np.asarray(keys, dtype=np.float32))
    Wq = np.ascontiguousarray(np.asarray(Wq, dtype=np.float32))
    Wk = np.ascontiguousarray(np.asarray(Wk, dtype=np.float32))
    v = np.ascontiguousarray(np.asarray(v, dtype=np.float32))

    nc = _get_nc()
    in_maps = []
    for c in range(N_CORES):
        sl = slice(c * B_LOC, (c + 1) * B_LOC)
        in_maps.append(
            {
                "query": query[sl],
                "keys": keys[sl],
                "Wq": Wq,
                "Wk": Wk,
                "v": v,
            }
        )
    last_err = None
    for attempt in range(3):
        try:
            res = run_bass_kernel_spmd(nc, in_maps, list(range(N_CORES)))
            out = np.concatenate(
                [np.asarray(res.results[c]["out"]) for c in range(N_CORES)], axis=0
            )
            break
        except Exception as e:  # transient device-unrecoverable states heal on retry
            last_err = e
            import time

            time.sleep(5)
    else:
        raise last_err
    return out.reshape(B, 1, H).astype(np.float32)


if __name__ == "__main__":
    rng = np.random.default_rng(0)
    q = rng.standard_normal((B, H), dtype=np.float32)
    k = rng.standard_normal((B, S, H), dtype=np.float32)
    wq = rng.standard_normal((H, A), dtype=np.float32) / np.sqrt(H)
    wk = rng.standard_normal((H, A), dtype=np.float32) / np.sqrt(H)
    vv = rng.standard_normal((A,), dtype=np.float32) / np.sqrt(A)
    o = kernel(query=q, keys=k, Wq=wq, Wk=wk, v=vv)
    print(o.shape, o.dtype)


# revision 15
# speedup vs baseline: 1.1068x; 1.1068x over previous
"""Bahdanau temporal attention on 8 Trainium2 NeuronCores.

Full-input contract: kernel(**inputs) takes the unsharded numpy arrays
(query (32,1024), keys (32,4096,1024), Wq (1024,512), Wk (1024,512),
v (512,)) and returns the full output (32,1,1024) float32.

Sharding: data-parallel over batch. Each of the 8 cores processes 4
batches; Wq/Wk/v are replicated. No collectives.

Per-core algorithm (B_loc=4, S=4096, H=1024, A=512):
  q_t    = query @ Wq                 (B_loc, A)
  k_t    = keys @ Wk                  (B_loc, S, A)
  energy = v . tanh(q_t + k_t)        (B_loc, S)
  w      = exp(energy)   (unnormalized; |energy| <= |v|_1 so exp cannot
                          overflow fp32 and no max-subtraction is needed)
  ctx    = (w @ keys) / sum(w)        (B_loc, H)

Pipeline per 512-row S-tile (s = r*128 + p), 3 software stages:
  stage A (front): SWDGE keys DMA casting f32->bf16 in flight, then
    xbar DMA-transpose -> keysT [h' (part), r, hc, s'],
  stage B (proj):
    - PE: k_t^T = Wk^T @ keys^T (bf16, f32 PSUM, N=512 streams),
    - ACT: T = tanh(k_t^T + q_t^T), q_t as per-partition bias,
    - PE: energy16 = v_rep16^T @ T -> psum [16, 512] (v_rep16 has 16
      identical columns of v, so all 16 psum rows hold the energy row),
    - ACT: w16 = exp(energy16) with accum_out -> per-tile softmax
      normalizer Z (no matmuls or max-subtraction needed),
    - xbar w16 [16, 512] -> wT4 [s' (part), r, 16] (same proven form as
      the q/v transposes),
  stage C (ctx, one stage later so the w xbar hides under stage B of
  the next tile):
    - PE: ctx [1, H] += wT4[:, r, 0].T @ keys_nat[:, r, :] -- full
      K=128 contraction, batched over r and the two H halves, psum
      accumulated across all 8 S-tiles of a batch.
Finalize per batch: Z = sum of per-tile normalizers (DVE adds),
reciprocal, scale the two psum halves, DMA out.
"""

import os
import sys

if "/opt/trn_rl_repo" not in sys.path:
    sys.path.insert(0, "/opt/trn_rl_repo")

import numpy as np

import concourse.bass as bass
import concourse.tile as tile
from concourse import bacc
from concourse import mybir
from concourse.bass_utils import run_bass_kernel_spmd

F32 = mybir.dt.float32
BF16 = mybir.dt.bfloat16

N_CORES = 8
B, S, H, A = 32, 4096, 1024, 512
B_LOC = B // N_CORES          # 4 batches per core
ST = 512                      # S-tile rows
N_ST = S // ST                # 8 S-tiles per batch
P = 128                       # partitions
HC = H // P                   # 8 contraction chunks
AC = A // P                   # 4 a-chunks
R = ST // P                   # 4 s-rows per partition (s = r*128 + p)

XBAR_SPLIT = int(os.environ.get("K_XBAR_SPLIT", "2"))  # xbar calls per keys tile
SWDGE_CAST = os.environ.get("K_SWDGE_CAST", "1") == "1"  # cast f32->bf16 in DMA

Tanh = mybir.ActivationFunctionType.Tanh
Exp = mybir.ActivationFunctionType.Exp


def build_bass():
    nc = bacc.Bacc()

    d_query = nc.declare_dram_parameter("query", [B_LOC, H], F32, isOutput=False)
    d_keys = nc.declare_dram_parameter("keys", [B_LOC, S, H], F32, isOutput=False)
    d_wq = nc.declare_dram_parameter("Wq", [H, A], F32, isOutput=False)
    d_wk = nc.declare_dram_parameter("Wk", [H, A], F32, isOutput=False)
    d_v = nc.declare_dram_parameter("v", [A], F32, isOutput=False)
    d_out = nc.declare_dram_parameter("out", [B_LOC, H], F32, isOutput=True)

    from contextlib import ExitStack

    with tile.TileContext(nc) as tc, ExitStack() as ctx:
        build_kernel_body(tc, d_query, d_keys, d_wq, d_wk, d_v, d_out, ctx)
    nc.compile()
    return nc


def build_kernel_body(tc, d_query, d_keys, d_wq, d_wk, d_v, d_out, ctx):
    nc = tc.nc

    consts = ctx.enter_context(tc.tile_pool(name="consts", bufs=1))
    keyp = ctx.enter_context(tc.tile_pool(name="keyp", bufs=6))
    keypf = ctx.enter_context(tc.tile_pool(name="keypf", bufs=3))
    keytp = ctx.enter_context(tc.tile_pool(name="keytp", bufs=5))
    tp = ctx.enter_context(tc.tile_pool(name="tp", bufs=3))
    wp = ctx.enter_context(tc.tile_pool(name="wp", bufs=5))
    wtp = ctx.enter_context(tc.tile_pool(name="wtp", bufs=5))
    bp = ctx.enter_context(tc.tile_pool(name="bp", bufs=2))
    smalls = ctx.enter_context(tc.tile_pool(name="smalls", bufs=2))
    pp_kt = ctx.enter_context(tc.tile_pool(name="pp_kt", bufs=3, space="PSUM"))
    pp_e = ctx.enter_context(tc.tile_pool(name="pp_e", bufs=1, space="PSUM"))
    pp_ctx = ctx.enter_context(tc.tile_pool(name="pp_ctx", bufs=4, space="PSUM"))

    # ---- constants ----
    # Wk in bf16, laid out [h' (part), hc, a]; SWDGE casts f32 -> bf16
    wk_bf = consts.tile([P, HC, A], BF16)
    nc.gpsimd.dma_start(out=wk_bf, in_=d_w_rearr(d_wk))
    wq_sb = consts.tile([P, HC, A], BF16)
    nc.gpsimd.dma_start(out=wq_sb, in_=d_w_rearr(d_wq))

    # v: load f32, DVE-cast into row 0 of a 16-row tile (single-producer
    # funnel so the xbar transpose carries only one wait), then xbar.
    v_f32 = consts.tile([1, A], F32)
    nc.gpsimd.dma_start(out=v_f32, in_=d_v[None, :])
    v16 = consts.tile([16, A], BF16)
    nc.vector.memset(v16, 0.0)
    nc.vector.tensor_copy(v16[0:1, :], v_f32)
    vT16 = consts.tile([P, AC, 16], BF16)
    nc.sync.dma_start(out=vT16, in_=v16, transpose=True)

    # query: same funnel pattern
    q_f32 = consts.tile([B_LOC, H], F32)
    nc.gpsimd.dma_start(out=q_f32, in_=d_query[:, :])
    q16 = consts.tile([16, H], BF16)
    nc.vector.memset(q16, 0.0)
    nc.vector.tensor_copy(q16[0:B_LOC, :], q_f32)
    qT16 = consts.tile([P, HC, 16], BF16)
    nc.sync.dma_start(out=qT16, in_=q16, transpose=True)

    # q_t = query @ Wq : psum (16, A), accumulate over hc
    ps_qt = pp_e.tile([16, A], F32, tag="e")
    for hc in range(HC):
        nc.tensor.matmul(
            ps_qt,
            lhsT=qT16[:, hc, :],
            rhs=wq_sb[:, hc, :],
            start=(hc == 0),
            stop=(hc == HC - 1),
        )
    qt16 = consts.tile([16, A], BF16)
    nc.vector.memset(qt16, 0.0)
    nc.vector.tensor_copy(qt16[0:B_LOC, :], ps_qt[0:B_LOC, :])
    # xbar -> qtT16 (128, AC, 16); tanh bias per (ac, b) = qtT16[:, ac, b]
    qtT16 = consts.tile([P, AC, 16], BF16)
    nc.sync.dma_start(out=qtT16, in_=qt16, transpose=True)

    # v_rep16[p, ac, m] = v[ac*128 + p] for m in 0..16: stationary operand
    # whose 16 identical columns replicate the energy row over 16 psum rows
    ones16 = consts.tile([P, 16], BF16)
    nc.vector.memset(ones16, 1.0)
    vTf = consts.tile([P, AC], F32)
    nc.vector.tensor_copy(vTf, vT16[:, :, 0])
    v_rep16 = consts.tile([P, AC, 16], BF16)
    for ac in range(AC):
        nc.vector.tensor_scalar_mul(v_rep16[:, ac, :], ones16, vTf[:, ac : ac + 1])

    # ---- main loop: 3-stage pipelined emission ----
    iters = [(b, st) for b in range(B_LOC) for st in range(N_ST)]
    loads = {}
    front = {}
    w16s = {}
    wts = {}
    ctx_psums = {}
    zparts = {}

    def stage_load(b, st):
        # keys tile [s' (part), r, h] with s = r*128 + p
        src = d_keys[b, st * ST : (st + 1) * ST, :].rearrange("(r p) h -> p r h", p=P)
        if SWDGE_CAST:
            keys_bf = keyp.tile([P, R, H], BF16, tag="keys")
            nc.gpsimd.dma_start(out=keys_bf, in_=src)
        else:
            keys_f32 = keypf.tile([P, R, H], F32, tag="keysf")
            nc.scalar.dma_start(out=keys_f32, in_=src)
            keys_bf = keyp.tile([P, R, H], BF16, tag="keys")
            nc.vector.tensor_copy(keys_bf, keys_f32)
        return keys_bf

    def stage_xpose(b, st):
        # xbar: keysT[h', r, hc, s'] = keys_bf[s', r, hc*128 + h']
        # NOTE: all DMA-transposes must stay on ONE queue -- concurrent
        # xbar transposes from different HWDGE queues corrupt each other
        # (shared xbar HW; the framework only serializes per-queue).
        keys_bf = loads[(b, st)]
        keysT = keytp.tile([P, R, HC, P], BF16, tag="kT")
        if XBAR_SPLIT == 1:
            nc.sync.dma_start(out=keysT, in_=keys_bf, transpose=True)
        else:
            rr = R // XBAR_SPLIT
            for j in range(XBAR_SPLIT):
                nc.sync.dma_start(
                    out=keysT[:, j * rr : (j + 1) * rr, :, :],
                    in_=keys_bf[:, j * rr : (j + 1) * rr, :],
                    transpose=True,
                )
        return keys_bf, keysT

    def stage_proj(b, st):
        keys_bf, keysT = front[(b, st)]
        if st == 0:
            z_parts = bp.tile([16, N_ST], F32, tag="z")
            zparts[b] = z_parts
        z_parts = zparts[b]

        # projection + tanh: T[a' (part), ac, s]
        T_sb = tp.tile([P, AC, ST], BF16, tag="T")
        for ac in range(AC):
            ps_kt = pp_kt.tile([P, ST], F32, tag="kt")
            for hc in range(HC):
                nc.tensor.matmul(
                    ps_kt,
                    lhsT=wk_bf[:, hc, ac * P : (ac + 1) * P],
                    rhs=keysT[:, :, hc, :],
                    start=(hc == 0),
                    stop=(hc == HC - 1),
                )
            nc.scalar.activation(
                T_sb[:, ac, :],
                ps_kt,
                Tanh,
                bias=qtT16[:, ac, b : b + 1],
            )

        # energy row, replicated over 16 psum rows
        ps_e = pp_e.tile([16, ST], F32, tag="e")
        for ac in range(AC):
            nc.tensor.matmul(
                ps_e,
                lhsT=v_rep16[:, ac, :],
                rhs=T_sb[:, ac, :],
                start=(ac == 0),
                stop=(ac == AC - 1),
            )

        # w16 = exp(energy); accum_out = per-tile softmax normalizer
        w16 = wp.tile([16, ST], BF16, tag="w")
        nc.scalar.activation(w16, ps_e, Exp, accum_out=z_parts[:, st : st + 1])
        w16s[(b, st)] = w16

    def stage_wx(b, st):
        # w transpose: wT4[p, r, i] = w[r*128 + p] for any i. Emitted two
        # steps after its exp so the in-order transpose queue never
        # head-of-line blocks on it.
        w16 = w16s.pop((b, st))
        wT4 = wtp.tile([P, R, 16], BF16, tag="wT")
        nc.sync.dma_start(out=wT4, in_=w16, transpose=True)
        wts[(b, st)] = wT4

    def stage_ctx(b, st):
        keys_bf, _ = front.pop((b, st))
        wT4 = wts.pop((b, st))
        first = st == 0
        last = st == N_ST - 1
        if first:
            ps_c0 = pp_ctx.tile([1, 512], F32, tag="ctx")
            ps_c1 = pp_ctx.tile([1, 512], F32, tag="ctx")
            ctx_psums[b] = (ps_c0, ps_c1)
        ps_c0, ps_c1 = ctx_psums[b]

        for r in range(R):
            nc.tensor.matmul(
                ps_c0,
                lhsT=wT4[:, r, 0:1],
                rhs=keys_bf[:, r, 0:512],
                start=(first and r == 0),
                stop=(last and r == R - 1),
            )
            nc.tensor.matmul(
                ps_c1,
                lhsT=wT4[:, r, 0:1],
                rhs=keys_bf[:, r, 512:1024],
                start=(first and r == 0),
                stop=(last and r == R - 1),
            )
        if last:
            finalize_batch(b, ps_c0, ps_c1)

    def finalize_batch(b, ps_c0, ps_c1):
        z_parts = zparts.pop(b)
        ctx_psums.pop(b)
        # Z = sum of the 8 per-tile normalizers (row 0 carries the value)
        za = smalls.tile([16, 1], F32, tag="za")
        zb = smalls.tile([16, 1], F32, tag="zb")
        nc.vector.tensor_scalar_add(za, z_parts[:, 0:1], z_parts[:, 1:2])
        nc.vector.tensor_scalar_add(zb, z_parts[:, 2:3], z_parts[:, 3:4])
        nc.vector.tensor_scalar_add(za, za, z_parts[:, 4:5])
        nc.vector.tensor_scalar_add(zb, zb, z_parts[:, 5:6])
        nc.vector.tensor_scalar_add(za, za, z_parts[:, 6:7])
        nc.vector.tensor_scalar_add(zb, zb, z_parts[:, 7:8])
        nc.vector.tensor_scalar_add(za, za, zb[0:16, 0:1])
        rz = smalls.tile([1, 1], F32, tag="rz")
        nc.vector.reciprocal(rz, za[0:1, 0:1])
        out_sb = smalls.tile([1, H], F32, tag="out")
        nc.vector.tensor_scalar_mul(out_sb[0:1, 0:512], ps_c0, rz)
        nc.vector.tensor_scalar_mul(out_sb[0:1, 512:1024], ps_c1, rz)
        nc.gpsimd.dma_start(out=d_out[b : b + 1, :], in_=out_sb)

    n = len(iters)
    for i in range(n + 4):
        if i < n:
            loads[iters[i]] = stage_load(*iters[i])
        if 1 <= i <= n:
            front[iters[i - 1]] = stage_xpose(*iters[i - 1])
            loads.pop(iters[i - 1])
        if 2 <= i <= n + 1:
            stage_proj(*iters[i - 2])
        if 3 <= i <= n + 2:
            stage_wx(*iters[i - 3])
        if 4 <= i <= n + 3:
            stage_ctx(*iters[i - 4])


def d_w_rearr(d_w):
    # (H, A) dram -> [h' (part), hc, a] view
    return d_w.rearrange("(hc p) a -> p hc a", p=P)


_CACHED_NC = None


def _get_nc():
    global _CACHED_NC
    if _CACHED_NC is None:
        _CACHED_NC = build_bass()
    return _CACHED_NC


def kernel(query, keys, Wq, Wk, v):
    query = np.ascontiguousarray(np.asarray(query, dtype=np.float32))
    keys = np.ascontiguousarray(np.asarray(keys, dtype=np.float32))
    Wq = np.ascontiguousarray(np.asarray(Wq, dtype=np.float32))
    Wk = np.ascontiguousarray(np.asarray(Wk, dtype=np.float32))
    v = np.ascontiguousarray(np.asarray(v, dtype=np.float32))

    nc = _get_nc()
    in_maps = []
    for c in range(N_CORES):
        sl = slice(c * B_LOC, (c + 1) * B_LOC)
        in_maps.append(
            {
                "query": query[sl],
                "keys": keys[sl],
                "Wq": Wq,
                "Wk": Wk,
                "v": v,
            }
        )
    last_err = None
    for attempt in range(3):
        try:
            res = run_bass_kernel_spmd(nc, in_maps, list(range(N_CORES)))
            out = np.concatenate(
                [np.asarray(res.results[c]["out"]) for c in range(N_CORES)], axis=0
            )
            break
        except Exception as e:  # transient device-unrecoverable states heal on retry
            last_err = e
            import time

            time.sleep(5)
    else:
        raise last_err
    return out.reshape(B, 1, H).astype(np.float32)


if __name__ == "__main__":
    rng = np.random.default_rng(0)
    q = rng.standard_normal((B, H), dtype=np.float32)
    k = rng.standard_normal((B, S, H), dtype=np.float32)
    wq = rng.standard_normal((H, A), dtype=np.float32) / np.sqrt(H)
    wk = rng.standard_normal((H, A), dtype=np.float32) / np.sqrt(H)
    vv = rng.standard_normal((A,), dtype=np.float32) / np.sqrt(A)
    o = kernel(query=q, keys=k, Wq=wq, Wk=wk, v=vv)
    print(o.shape, o.dtype)


# revision 20
# speedup vs baseline: 1.3488x; 1.2187x over previous
"""Bahdanau temporal attention on 8 Trainium2 NeuronCores.

Full-input contract: kernel(**inputs) takes the unsharded numpy arrays
(query (32,1024), keys (32,4096,1024), Wq (1024,512), Wk (1024,512),
v (512,)) and returns the full output (32,1,1024) float32.

Sharding: data-parallel over batch. Each of the 8 cores processes 4
batches; Wq/Wk/v are replicated. No collectives.

Per-core algorithm (B_loc=4, S=4096, H=1024, A=512):
  q_t    = query @ Wq                 (B_loc, A)
  k_t    = keys @ Wk                  (B_loc, S, A)
  energy = v . tanh(q_t + k_t)        (B_loc, S)
  w      = exp(energy)   (unnormalized; |energy| <= |v|_1 so exp cannot
                          overflow fp32 and no max-subtraction is needed)
  ctx    = (w @ keys) / sum(w)        (B_loc, H)

Pipeline per 512-row S-tile (s = r*128 + p), 3 software stages:
  stage A (front): SWDGE keys DMA casting f32->bf16 in flight, then
    xbar DMA-transpose -> keysT [h' (part), r, hc, s'],
  stage B (proj):
    - PE: k_t^T = Wk^T @ keys^T (bf16, f32 PSUM, N=512 streams),
    - ACT: T = tanh(k_t^T + q_t^T), q_t as per-partition bias,
    - PE: energy16 = v_rep16^T @ T -> psum [16, 512] (v_rep16 has 16
      identical columns of v, so all 16 psum rows hold the energy row),
    - ACT: w16 = exp(energy16) with accum_out -> per-tile softmax
      normalizer Z (no matmuls or max-subtraction needed),
    - xbar w16 [16, 512] -> wT4 [s' (part), r, 16] (same proven form as
      the q/v transposes),
  stage C (ctx, one stage later so the w xbar hides under stage B of
  the next tile):
    - PE: ctx [1, H] += wT4[:, r, 0].T @ keys_nat[:, r, :] -- full
      K=128 contraction, batched over r and the two H halves, psum
      accumulated across all 8 S-tiles of a batch.
Finalize per batch: Z = sum of per-tile normalizers (DVE adds),
reciprocal, scale the two psum halves, DMA out.
"""

import os
import sys

if "/opt/trn_rl_repo" not in sys.path:
    sys.path.insert(0, "/opt/trn_rl_repo")

import numpy as np

import concourse.bass as bass
import concourse.tile as tile
from concourse import bacc
from concourse import mybir
from concourse.bass_utils import run_bass_kernel_spmd

F32 = mybir.dt.float32
BF16 = mybir.dt.bfloat16

N_CORES = 8
B, S, H, A = 32, 4096, 1024, 512
B_LOC = B // N_CORES          # 4 batches per core
ST = 512                      # S-tile rows
N_ST = S // ST                # 8 S-tiles per batch
P = 128                       # partitions
HC = H // P                   # 8 contraction chunks
AC = A // P                   # 4 a-chunks
R = ST // P                   # 4 s-rows per partition (s = r*128 + p)

XBAR_SPLIT = int(os.environ.get("K_XBAR_SPLIT", "2"))  # xbar calls per keys tile
SWDGE_CAST = os.environ.get("K_SWDGE_CAST", "1") == "1"  # cast f32->bf16 in DMA

Tanh = mybir.ActivationFunctionType.Tanh
Exp = mybir.ActivationFunctionType.Exp


def build_bass():
    nc = bacc.Bacc()

    d_query = nc.declare_dram_parameter("query", [B_LOC, H], F32, isOutput=False)
    d_keys = nc.declare_dram_parameter("keys", [B_LOC, S, H], F32, isOutput=False)
    d_wq = nc.declare_dram_parameter("Wq", [H, A], F32, isOutput=False)
    d_wk = nc.declare_dram_parameter("Wk", [H, A], F32, isOutput=False)
    d_v = nc.declare_dram_parameter("v", [A], F32, isOutput=False)
    d_out = nc.declare_dram_parameter("out", [B_LOC, H], F32, isOutput=True)

    from contextlib import ExitStack

    with tile.TileContext(nc) as tc, ExitStack() as ctx:
        build_kernel_body(tc, d_query, d_keys, d_wq, d_wk, d_v, d_out, ctx)
    nc.compile()
    return nc


def build_kernel_body(tc, d_query, d_keys, d_wq, d_wk, d_v, d_out, ctx):
    nc = tc.nc

    consts = ctx.enter_context(tc.tile_pool(name="consts", bufs=1))
    keyp = ctx.enter_context(tc.tile_pool(name="keyp", bufs=4))
    keytp = ctx.enter_context(tc.tile_pool(name="keytp", bufs=3))
    tp = ctx.enter_context(tc.tile_pool(name="tp", bufs=3))
    wp = ctx.enter_context(tc.tile_pool(name="wp", bufs=3))
    wtp = ctx.enter_context(tc.tile_pool(name="wtp", bufs=3))
    bp = ctx.enter_context(tc.tile_pool(name="bp", bufs=2))
    smalls = ctx.enter_context(tc.tile_pool(name="smalls", bufs=2))
    pp_kt = ctx.enter_context(tc.tile_pool(name="pp_kt", bufs=3, space="PSUM"))
    pp_e = ctx.enter_context(tc.tile_pool(name="pp_e", bufs=1, space="PSUM"))
    pp_ctx = ctx.enter_context(tc.tile_pool(name="pp_ctx", bufs=4, space="PSUM"))

    # ---- constants ----
    # Wk in bf16, laid out [h' (part), hc, a]; SWDGE casts f32 -> bf16
    wk_bf = consts.tile([P, HC, A], BF16)
    nc.gpsimd.dma_start(out=wk_bf, in_=d_w_rearr(d_wk))
    wq_sb = consts.tile([P, HC, A], BF16)
    nc.gpsimd.dma_start(out=wq_sb, in_=d_w_rearr(d_wq))

    # v: load f32, DVE-cast into row 0 of a 16-row tile (single-producer
    # funnel so the xbar transpose carries only one wait), then xbar.
    v_f32 = consts.tile([1, A], F32)
    nc.gpsimd.dma_start(out=v_f32, in_=d_v[None, :])
    v16 = consts.tile([16, A], BF16)
    nc.vector.memset(v16, 0.0)
    nc.vector.tensor_copy(v16[0:1, :], v_f32)
    vT16 = consts.tile([P, AC, 16], BF16)
    nc.sync.dma_start(out=vT16, in_=v16, transpose=True)

    # query: same funnel pattern
    q_f32 = consts.tile([B_LOC, H], F32)
    nc.gpsimd.dma_start(out=q_f32, in_=d_query[:, :])
    q16 = consts.tile([16, H], BF16)
    nc.vector.memset(q16, 0.0)
    nc.vector.tensor_copy(q16[0:B_LOC, :], q_f32)
    qT16 = consts.tile([P, HC, 16], BF16)
    nc.sync.dma_start(out=qT16, in_=q16, transpose=True)

    # q_t = query @ Wq : psum (16, A), accumulate over hc
    ps_qt = pp_e.tile([16, A], F32, tag="e")
    for hc in range(HC):
        nc.tensor.matmul(
            ps_qt,
            lhsT=qT16[:, hc, :],
            rhs=wq_sb[:, hc, :],
            start=(hc == 0),
            stop=(hc == HC - 1),
        )
    qt16 = consts.tile([16, A], BF16)
    nc.vector.memset(qt16, 0.0)
    nc.vector.tensor_copy(qt16[0:B_LOC, :], ps_qt[0:B_LOC, :])
    # xbar -> qtT16 (128, AC, 16); tanh bias per (ac, b) = qtT16[:, ac, b]
    qtT16 = consts.tile([P, AC, 16], BF16)
    nc.sync.dma_start(out=qtT16, in_=qt16, transpose=True)

    # v_rep16[p, ac, m] = v[ac*128 + p] for m in 0..16: stationary operand
    # whose 16 identical columns replicate the energy row over 16 psum rows
    ones16 = consts.tile([P, 16], BF16)
    nc.vector.memset(ones16, 1.0)
    vTf = consts.tile([P, AC], F32)
    nc.vector.tensor_copy(vTf, vT16[:, :, 0])
    v_rep16 = consts.tile([P, AC, 16], BF16)
    for ac in range(AC):
        nc.vector.tensor_scalar_mul(v_rep16[:, ac, :], ones16, vTf[:, ac : ac + 1])

    # ---- main loop ----
    # DMA is batched at 2-tile (1024 s-row) "round" granularity so the
    # whole pipeline issues only ~2 DMAs per compute tile: the framework
    # has just 8 DMA-completion semaphore lanes shared by every DMA, and
    # each new DMA must wait for the lane's previous user, so in-flight
    # depth = 8 / (DMAs per tile). At 4 DMAs/tile that window (2 tiles)
    # could not hide the load+transpose latency and the pipeline ran in
    # lockstep (~18 us/tile).
    R2 = 2 * R                 # 8 s-rows per partition per round
    N_RD = B_LOC * N_ST // 2   # 16 rounds, 2 compute tiles each
    loads = {}
    front = {}
    w16s = {}
    wts = {}
    ctx_psums = {}
    zparts = {}

    def stage_load(g):
        b, h2 = divmod(g, N_ST // 2)
        # keys round tile [s' (part), r, h] with s = r*128 + p (r in 0..8)
        src = d_keys[b, h2 * 2 * ST : (h2 + 1) * 2 * ST, :].rearrange(
            "(r p) h -> p r h", p=P
        )
        keys_bf = keyp.tile([P, R2, H], BF16, tag="keys")
        nc.gpsimd.dma_start(out=keys_bf, in_=src)
        loads[g] = keys_bf

    def stage_xpose(g):
        # xbar: keysT[h', r, hc, s'] = keys_bf[s', r, hc*128 + h']
        # NOTE: all DMA-transposes stay on ONE queue -- concurrent xbar
        # transposes from different HWDGE queues corrupt each other
        # (shared xbar HW; the framework only serializes per-queue).
        keys_bf = loads.pop(g)
        keysT = keytp.tile([P, R2, HC, P], BF16, tag="kT")
        for j in range(2):
            nc.sync.dma_start(
                out=keysT[:, j * R : (j + 1) * R, :, :],
                in_=keys_bf[:, j * R : (j + 1) * R, :],
                transpose=True,
            )
        front[g] = (keys_bf, keysT)

    def stage_proj(g, j):
        b, h2 = divmod(g, N_ST // 2)
        st = 2 * h2 + j
        keys_bf, keysT = front[g]
        if st == 0:
            z_new = bp.tile([16, N_ST], F32, tag="z")
            zparts[b] = z_new
        z_parts = zparts[b]

        # projection + tanh: T[a' (part), ac, s]
        T_sb = tp.tile([P, AC, ST], BF16, tag="T")
        for ac in range(AC):
            ps_kt = pp_kt.tile([P, ST], F32, tag="kt")
            for hc in range(HC):
                nc.tensor.matmul(
                    ps_kt,
                    lhsT=wk_bf[:, hc, ac * P : (ac + 1) * P],
                    rhs=keysT[:, j * R : (j + 1) * R, hc, :],
                    start=(hc == 0),
                    stop=(hc == HC - 1),
                )
            nc.scalar.activation(
                T_sb[:, ac, :],
                ps_kt,
                Tanh,
                bias=qtT16[:, ac, b : b + 1],
            )

        # energy row, replicated over 16 psum rows
        ps_e = pp_e.tile([16, ST], F32, tag="e")
        for ac in range(AC):
            nc.tensor.matmul(
                ps_e,
                lhsT=v_rep16[:, ac, :],
                rhs=T_sb[:, ac, :],
                start=(ac == 0),
                stop=(ac == AC - 1),
            )

        # w16 half j = exp(energy); accum_out = per-tile normalizer
        if j == 0:
            w_new = wp.tile([16, 2, ST], BF16, tag="w")
            w16s[g] = w_new
        w16 = w16s[g]
        nc.scalar.activation(
            w16[:, j, :], ps_e, Exp, accum_out=z_parts[:, st : st + 1]
        )

    def stage_wx(g):
        # one paired w transpose per round: wT4x[p, 4j+r, i] = w half j at
        # s = r*128 + p, any i. Emitted well after both exps so the
        # in-order transpose queue never head-of-line blocks on it.
        w16 = w16s.pop(g)
        wT4x = wtp.tile([P, R2, 16], BF16, tag="wT")
        nc.sync.dma_start(out=wT4x, in_=w16, transpose=True)
        wts[g] = wT4x

    def stage_ctx(g, j):
        b, h2 = divmod(g, N_ST // 2)
        st = 2 * h2 + j
        keys_bf, _ = front[g]
        wT4x = wts[g]
        if j == 1:
            front.pop(g)
            wts.pop(g)
        first = st == 0
        last = st == N_ST - 1
        if first:
            ps_c0 = pp_ctx.tile([1, 512], F32, tag="ctx")
            ps_c1 = pp_ctx.tile([1, 512], F32, tag="ctx")
            ctx_psums[b] = (ps_c0, ps_c1)
        ps_c0, ps_c1 = ctx_psums[b]

        for r in range(R):
            nc.tensor.matmul(
                ps_c0,
                lhsT=wT4x[:, j * R + r, 0:1],
                rhs=keys_bf[:, j * R + r, 0:512],
                start=(first and r == 0),
                stop=(last and r == R - 1),
            )
            nc.tensor.matmul(
                ps_c1,
                lhsT=wT4x[:, j * R + r, 0:1],
                rhs=keys_bf[:, j * R + r, 512:1024],
                start=(first and r == 0),
                stop=(last and r == R - 1),
            )
        if last:
            finalize_batch(b, ps_c0, ps_c1)

    def finalize_batch(b, ps_c0, ps_c1):
        z_parts = zparts.pop(b)
        ctx_psums.pop(b)
        # Z = sum of the 8 per-tile normalizers (row 0 carries the value)
        za = smalls.tile([16, 1], F32, tag="za")
        zb = smalls.tile([16, 1], F32, tag="zb")
        nc.vector.tensor_scalar_add(za, z_parts[:, 0:1], z_parts[:, 1:2])
        nc.vector.tensor_scalar_add(zb, z_parts[:, 2:3], z_parts[:, 3:4])
        nc.vector.tensor_scalar_add(za, za, z_parts[:, 4:5])
        nc.vector.tensor_scalar_add(zb, zb, z_parts[:, 5:6])
        nc.vector.tensor_scalar_add(za, za, z_parts[:, 6:7])
        nc.vector.tensor_scalar_add(zb, zb, z_parts[:, 7:8])
        nc.vector.tensor_scalar_add(za, za, zb[0:16, 0:1])
        rz = smalls.tile([1, 1], F32, tag="rz")
        nc.vector.reciprocal(rz, za[0:1, 0:1])
        out_sb = smalls.tile([1, H], F32, tag="out")
        nc.vector.tensor_scalar_mul(out_sb[0:1, 0:512], ps_c0, rz)
        nc.vector.tensor_scalar_mul(out_sb[0:1, 512:1024], ps_c1, rz)
        nc.gpsimd.dma_start(out=d_out[b : b + 1, :], in_=out_sb)

    for i in range(N_RD + 3):
        if i < N_RD:
            stage_load(i)
        if 1 <= i <= N_RD:
            stage_xpose(i - 1)
        if 2 <= i <= N_RD + 1:
            stage_proj(i - 2, 0)
            stage_proj(i - 2, 1)
        if 3 <= i <= N_RD + 2:
            stage_wx(i - 3)
            stage_ctx(i - 3, 0)
            stage_ctx(i - 3, 1)


def d_w_rearr(d_w):
    # (H, A) dram -> [h' (part), hc, a] view
    return d_w.rearrange("(hc p) a -> p hc a", p=P)


_CACHED_NC = None


def _get_nc():
    global _CACHED_NC
    if _CACHED_NC is None:
        _CACHED_NC = build_bass()
    return _CACHED_NC


def kernel(query, keys, Wq, Wk, v):
    query = np.ascontiguousarray(np.asarray(query, dtype=np.float32))
    keys = np.ascontiguousarray(np.asarray(keys, dtype=np.float32))
    Wq = np.ascontiguousarray(np.asarray(Wq, dtype=np.float32))
    Wk = np.ascontiguousarray(np.asarray(Wk, dtype=np.float32))
    v = np.ascontiguousarray(np.asarray(v, dtype=np.float32))

    nc = _get_nc()
    in_maps = []
    for c in range(N_CORES):
        sl = slice(c * B_LOC, (c + 1) * B_LOC)
        in_maps.append(
            {
                "query": query[sl],
                "keys": keys[sl],
                "Wq": Wq,
                "Wk": Wk,
                "v": v,
            }
        )
    last_err = None
    for attempt in range(3):
        try:
            res = run_bass_kernel_spmd(nc, in_maps, list(range(N_CORES)))
            out = np.concatenate(
                [np.asarray(res.results[c]["out"]) for c in range(N_CORES)], axis=0
            )
            break
        except Exception as e:  # transient device-unrecoverable states heal on retry
            last_err = e
            import time

            time.sleep(5)
    else:
        raise last_err
    return out.reshape(B, 1, H).astype(np.float32)


if __name__ == "__main__":
    rng = np.random.default_rng(0)
    q = rng.standard_normal((B, H), dtype=np.float32)
    k = rng.standard_normal((B, S, H), dtype=np.float32)
    wq = rng.standard_normal((H, A), dtype=np.float32) / np.sqrt(H)
    wk = rng.standard_normal((H, A), dtype=np.float32) / np.sqrt(H)
    vv = rng.standard_normal((A,), dtype=np.float32) / np.sqrt(A)
    o = kernel(query=q, keys=k, Wq=wq, Wk=wk, v=vv)
    print(o.shape, o.dtype)
